# revision 1
# baseline (speedup 1.0000x reference)
"""MDRNN 2D-grid recurrence kernel for 8 Trainium2 NeuronCores.

h[i,j] = tanh(x[i,j] @ w + h[i-1,j]*u0 + h[i,j-1]*u1 + bias)

Strategy:
  - Data-parallel over batch: B=16 -> 2 batch elements per core.
  - Host pre-transposes x into diag-ordered [SIN+1, cells*b] layout
    (ones row appended so the GEMM also adds the bias), so the kernel
    needs no on-chip transpose and the wavefront walks contiguous
    slices.
  - On chip, layout A: SOUT on partitions. Per anti-diagonal d:
      1 matmul (w stationary) -> a' in PSUM
      2 fused scalar_tensor_tensor ops on DVE:
          t1 = h_left*u1 + a'   ;   z = h_up*u0 + t1
      1 ACT tanh -> ring buffer, DMA'd out diag-packed.
  - Ring of 4 zero-initialized i-aligned buffers gives the recurrence
    boundary zeros for free.
  - Host inverse-permutes the diag-packed output back to (i,j,b,o).
"""

import numpy as np

D1, D2, B, SIN, SOUT = 128, 128, 16, 64, 128
NCORES = 8
BLOC = B // NCORES  # 2
NCELLS = D1 * D2
NCOLS = NCELLS * BLOC  # 32768
ND = D1 + D2 - 1  # 255


def _diag_order():
    I, J, bases = [], [], [0]
    for d in range(ND):
        i0 = max(0, d - (D2 - 1))
        i1 = min(D1 - 1, d)
        for i in range(i0, i1 + 1):
            I.append(i)
            J.append(d - i)
        bases.append(len(I))
    return np.array(I), np.array(J), np.array(bases)


_CACHE = {}


def _build_program():
    if "nc" in _CACHE:
        return _CACHE["nc"]
    import concourse.mybir as mybir
    from concourse import bacc
    import concourse.bass as bass
    from concourse.tile import TileContext

    f32 = mybir.dt.float32
    mult = mybir.AluOpType.mult
    add = mybir.AluOpType.add
    Tanh = mybir.ActivationFunctionType.Tanh

    _, _, bases = _diag_order()

    nc = bacc.Bacc(None, target_bir_lowering=False)
    xa = nc.dram_tensor("xa", (SIN + 1, NCOLS), f32, kind="ExternalInput")
    wb = nc.dram_tensor("wb", (SIN + 1, SOUT), f32, kind="ExternalInput")
    uu = nc.dram_tensor("uu", (SOUT, 2), f32, kind="ExternalInput")
    ho = nc.dram_tensor("ho", (SOUT, NCOLS), f32, kind="ExternalOutput")

    K = 4  # ring depth
    RW = (D1 + 1) * BLOC  # 258: slots for i = -1..127, b-pairs

    with TileContext(nc) as tc:
        with (
            tc.tile_pool(name="const", bufs=1) as constp,
            tc.tile_pool(name="xbig", bufs=1) as xbigp,
            tc.tile_pool(name="ring", bufs=1) as ringp,
            tc.tile_pool(name="scratch", bufs=4) as scrp,
            tc.tile_pool(name="psum", bufs=8, space=bass.MemorySpace.PSUM) as psump,
        ):
            wb_sb = constp.tile([SIN + 1, SOUT], f32, tag="wb")
            nc.sync.dma_start(wb_sb[:], wb[:])
            u_sb = constp.tile([SOUT, 2], f32, tag="uu")
            nc.sync.dma_start(u_sb[:], uu[:])
            u0 = u_sb[:, 0:1]
            u1 = u_sb[:, 1:2]

            xa_sb = xbigp.tile([SIN + 1, NCOLS], f32, tag="xa")
            nchunk = 16
            csz = NCOLS // nchunk
            for k in range(nchunk):
                nc.sync.dma_start(
                    xa_sb[:, k * csz : (k + 1) * csz],
                    xa[:, k * csz : (k + 1) * csz],
                )

            rings = []
            for m in range(K):
                t = ringp.tile([SOUT, RW], f32, tag=f"ring{m}")
                nc.vector.memset(t[:], 0.0)
                rings.append(t)

            for d in range(ND):
                i0 = max(0, d - (D2 - 1))
                i1 = min(D1 - 1, d)
                C = i1 - i0 + 1
                n = C * BLOC
                base = int(bases[d]) * BLOC
                prev = rings[(d - 1) % K]
                cur = rings[d % K]

                ps = psump.tile([SOUT, 256], f32, tag="ps")
                nc.tensor.matmul(
                    out=ps[:, :n],
                    lhsT=wb_sb[:],
                    rhs=xa_sb[:, base : base + n],
                    start=True,
                    stop=True,
                )
                t1 = scrp.tile([SOUT, 256], f32, tag="t1")
                # t1 = h_left * u1 + a'
                nc.vector.scalar_tensor_tensor(
                    out=t1[:, :n],
                    in0=prev[:, (i0 + 1) * BLOC : (i1 + 2) * BLOC],
                    scalar=u1,
                    in1=ps[:, :n],
                    op0=mult,
                    op1=add,
                )
                t2 = scrp.tile([SOUT, 256], f32, tag="t2")
                # z = h_up * u0 + t1
                nc.vector.scalar_tensor_tensor(
                    out=t2[:, :n],
                    in0=prev[:, i0 * BLOC : (i1 + 1) * BLOC],
                    scalar=u0,
                    in1=t1[:, :n],
                    op0=mult,
                    op1=add,
                )
                nc.scalar.activation(
                    out=cur[:, (i0 + 1) * BLOC : (i1 + 2) * BLOC],
                    in_=t2[:, :n],
                    func=Tanh,
                )
                nc.sync.dma_start(
                    ho[:, base : base + n],
                    cur[:, (i0 + 1) * BLOC : (i1 + 2) * BLOC],
                )

    nc.compile()
    _CACHE["nc"] = nc
    return nc


def _prep_inputs(x, w, u, bias):
    I, J, _ = _diag_order()
    xa_cells = np.ascontiguousarray(x[I, J])  # (16384, B, SIN)
    wbm = np.concatenate([w, bias[None, :]], axis=0).astype(np.float32)  # (65,128)
    um = np.ascontiguousarray(u.T).astype(np.float32)  # (128, 2): col0=u0, col1=u1
    in_maps = []
    for c in range(NCORES):
        xc = xa_cells[:, c * BLOC : (c + 1) * BLOC, :]  # (16384, 2, 64)
        xc = xc.transpose(2, 0, 1).reshape(SIN, NCOLS)  # (64, 32768)
        xc = np.concatenate([xc, np.ones((1, NCOLS), np.float32)], axis=0)
        in_maps.append(
            {"xa": np.ascontiguousarray(xc), "wb": wbm, "uu": um}
        )
    return in_maps


def _assemble(results):
    I, J, _ = _diag_order()
    out = np.zeros((D1, D2, B, SOUT), np.float32)
    for c in range(NCORES):
        hoc = results[c]["ho"]  # (128, 32768)
        h_core = hoc.reshape(SOUT, NCELLS, BLOC).transpose(1, 2, 0)
        out[I, J, c * BLOC : (c + 1) * BLOC, :] = h_core
    return out


def kernel(x, w, u, bias, _trace=False):
    from concourse.bass_utils import run_bass_kernel_spmd

    x = np.asarray(x, dtype=np.float32)
    w = np.asarray(w, dtype=np.float32)
    u = np.asarray(u, dtype=np.float32)
    bias = np.asarray(bias, dtype=np.float32)

    nc = _build_program()
    in_maps = _prep_inputs(x, w, u, bias)
    res = run_bass_kernel_spmd(
        nc, in_maps, core_ids=list(range(NCORES)), trace=_trace
    )
    _CACHE["last_result"] = res
    return _assemble(res.results)
